# revision 9
# baseline (speedup 1.0000x reference)
"""Trainium2 Bass kernel for windowed mean-pooled cosine-sim multi-head attention.

Math (matches reference):
  km[n,:]  = mean over 16384 windows of window-partitioned k (n in 0..15), same for v
  kn       = l2norm(km per head-slice of 32 ch)
  t[i,h]   = scale_h / ||q[i, head h]||          (scale_h = exp(min(logit_scale, ln 100)))
  s[i,h,n] = t[i,h] * (q[i,h,:] . kn[h,n,:]) + mask[i,n]
  out[i, h-slice] = softmax_n(s[i,h,:]) @ vm[h,n,:]

Sharding: q/mask/out row-sharded 8 ways; k/v row-sharded for the mean
reduction with an AllReduce of the tiny [32,128] partial means.

Host-side prep: q and mask are transposed on host (channel-major) so the
device can DMA channel-major tiles directly (f32 DMA-transpose is not
supported by HW); all FLOPs and all large-tensor streaming happen on device.
"""

import math
import sys

import numpy as np

sys.path.insert(0, "/opt/trn_rl_repo")

NCORES = 8
Q_TOTAL = 262144
C = 128
NH = 4
HD = 32
N = 16  # window size 4x4
F = 512  # supertile: 512 q rows (4 x 128) / 512 k rows per step
LN100 = math.log(100.0)


def build_kernel(nc, rows, ncores, nwin_total, use_collective):
    """Trace the per-core kernel into `nc`. rows = per-core row count."""
    import concourse.bass as bass
    import concourse.tile as tile
    from concourse import mybir

    f32 = mybir.dt.float32
    AF = mybir.ActivationFunctionType
    ALU = mybir.AluOpType
    nst = rows // F
    assert rows % (4 * F) == 0, "need rows % 2048 == 0 so selector phase repeats"

    # ---- kernel I/O ----
    qT = nc.dram_tensor("qT", [C, rows], f32, kind="ExternalInput")
    mT = nc.dram_tensor("mT", [N, rows], f32, kind="ExternalInput")
    kin = nc.dram_tensor("kin", [rows, C], f32, kind="ExternalInput")
    vin = nc.dram_tensor("vin", [rows, C], f32, kind="ExternalInput")
    # host-precomputed constants
    sel_d = nc.dram_tensor("sel", [C, 4 * N], f32, kind="ExternalInput")
    onesblk_d = nc.dram_tensor("onesblk", [C, NH], f32, kind="ExternalInput")
    blkind_d = nc.dram_tensor("blkind", [NH, C], f32, kind="ExternalInput")
    repl16_d = nc.dram_tensor("repl16", [N, 4 * N], f32, kind="ExternalInput")
    ones16_d = nc.dram_tensor("ones16", [4 * N, NH], f32, kind="ExternalInput")
    ident_d = nc.dram_tensor("ident", [N, N], f32, kind="ExternalInput")
    lnscale_d = nc.dram_tensor("lnscale", [NH, 1], f32, kind="ExternalInput")
    negscale_d = nc.dram_tensor("negscale", [4 * N, 1], f32, kind="ExternalInput")
    out_d = nc.dram_tensor("out", [rows, C], f32, kind="ExternalOutput")

    with tile.TileContext(nc) as tc:
        with (
            tc.tile_pool(name="consts", bufs=1) as cpool,
            tc.tile_pool(name="smalls", bufs=1) as spool,
            tc.tile_pool(name="dram", bufs=1, space="DRAM") as dpool,
        ):
            # load constants
            sel = cpool.tile([C, 4 * N], f32)
            nc.sync.dma_start(out=sel, in_=sel_d[:, :])
            onesblk = cpool.tile([C, NH], f32)
            nc.sync.dma_start(out=onesblk, in_=onesblk_d[:, :])
            blkind = cpool.tile([NH, C], f32)
            nc.sync.dma_start(out=blkind, in_=blkind_d[:, :])
            repl16 = cpool.tile([N, 4 * N], f32)
            nc.sync.dma_start(out=repl16, in_=repl16_d[:, :])
            ones16 = cpool.tile([4 * N, NH], f32)
            nc.sync.dma_start(out=ones16, in_=ones16_d[:, :])
            ident = cpool.tile([N, N], f32)
            nc.sync.dma_start(out=ident, in_=ident_d[:, :])
            lnscale = cpool.tile([NH, 1], f32)
            nc.sync.dma_start(out=lnscale, in_=lnscale_d[:, :])
            negscale = cpool.tile([4 * N, 1], f32)
            nc.sync.dma_start(out=negscale, in_=negscale_d[:, :])

            # ---------------- phase KV: windowed means ----------------
            with (
                tc.tile_pool(name="kv", bufs=6) as kvpool,
                tc.tile_pool(name="pkv", bufs=1, space="PSUM") as pkv,
            ):
                kmP = pkv.tile([N, C], f32, tag="km")
                vmP = pkv.tile([N, C], f32, tag="vm")
                for j in range(nst):
                    src_k = kin[j * F : (j + 1) * F, :].rearrange(
                        "(s p) c -> p s c", p=128
                    )
                    kst = kvpool.tile([128, 4, 128], f32, tag="kst")
                    nc.sync.dma_start(out=kst, in_=src_k)
                    src_v = vin[j * F : (j + 1) * F, :].rearrange(
                        "(s p) c -> p s c", p=128
                    )
                    vst = kvpool.tile([128, 4, 128], f32, tag="vst")
                    nc.sync.dma_start(out=vst, in_=src_v)
                    a = j % 4
                    first = j == 0
                    last = j == nst - 1
                    for s in range(4):
                        nc.tensor.matmul(
                            kmP[:, :],
                            sel[:, N * a : N * (a + 1)],
                            kst[:, s, :],
                            start=(first and s == 0),
                            stop=(last and s == 3),
                        )
                        nc.tensor.matmul(
                            vmP[:, :],
                            sel[:, N * a : N * (a + 1)],
                            vst[:, s, :],
                            start=(first and s == 0),
                            stop=(last and s == 3),
                        )
                km_part = spool.tile([N, C], f32)
                nc.scalar.copy(out=km_part, in_=kmP[:, :])
                vm_part = spool.tile([N, C], f32)
                nc.scalar.copy(out=vm_part, in_=vmP[:, :])

            if use_collective:
                cc_in = dpool.tile([2 * N, C], f32)
                cc_out = dpool.tile([2 * N, C], f32)
                nc.gpsimd.dma_start(out=cc_in[0:N, :], in_=km_part[:, :])
                nc.gpsimd.dma_start(out=cc_in[N : 2 * N, :], in_=vm_part[:, :])
                nc.gpsimd.collective_compute(
                    "AllReduce",
                    mybir.AluOpType.add,
                    replica_groups=[list(range(ncores))],
                    ins=[cc_in.opt()],
                    outs=[cc_out.opt()],
                )
                km = spool.tile([N, C], f32)
                nc.sync.dma_start(out=km, in_=cc_out[0:N, :])
                vm = spool.tile([N, C], f32)
                nc.sync.dma_start(out=vm, in_=cc_out[N : 2 * N, :])
            else:
                km, vm = km_part, vm_part

            # ---------------- build knb (normalized kT, block-diag) & vmap ----
            kms = spool.tile([N, C], f32)
            nc.scalar.activation(out=kms, in_=km[:, :], func=AF.Square)
            ssk = spool.tile([N, NH], f32)
            nc.vector.reduce_sum(
                out=ssk,
                in_=kms[:, :].rearrange("p (h c) -> p h c", h=NH),
                axis=mybir.AxisListType.X,
            )
            sqk = spool.tile([N, NH], f32)
            nc.scalar.activation(out=sqk, in_=ssk[:, :], func=AF.Sqrt)
            tK = spool.tile([N, NH], f32)
            nc.vector.reciprocal(out=tK, in_=sqk[:, :])
            kn = spool.tile([N, C], f32)
            for h in range(NH):
                nc.vector.tensor_scalar_mul(
                    kn[:, HD * h : HD * (h + 1)],
                    km[:, HD * h : HD * (h + 1)],
                    tK[:, h : h + 1],
                )
            knb = cpool.tile([C, 4 * N], f32)
            nc.gpsimd.memset(knb[:, :], 0.0)
            with tc.tile_pool(name="pknb", bufs=1, space="PSUM") as pknb:
                knbP = pknb.tile([HD, 4 * N], f32)
                for h in range(NH):
                    nc.tensor.transpose(
                        knbP[:, N * h : N * (h + 1)],
                        kn[:, HD * h : HD * (h + 1)],
                        ident[:, :],
                    )
                knb_stage = spool.tile([HD, 4 * N], f32)
                nc.scalar.copy(out=knb_stage, in_=knbP[:, :])
                for h in range(NH):
                    nc.sync.dma_start(
                        out=knb[HD * h : HD * (h + 1), N * h : N * (h + 1)],
                        in_=knb_stage[:, N * h : N * (h + 1)],
                    )
            vmap = cpool.tile([4 * N, C], f32)
            nc.gpsimd.memset(vmap[:, :], 0.0)
            for h in range(NH):
                nc.sync.dma_start(
                    out=vmap[N * h : N * (h + 1), HD * h : HD * (h + 1)],
                    in_=vm[:, HD * h : HD * (h + 1)],
                )

            # ---------------- phase Q ----------------
            with (
                tc.tile_pool(name="qp", bufs=4) as qpool,
                tc.tile_pool(name="mp", bufs=4) as mpool,
                tc.tile_pool(name="q2p", bufs=2) as q2pool,
                tc.tile_pool(name="lp", bufs=2) as lpool,
                tc.tile_pool(name="tp", bufs=2) as tpool,
                tc.tile_pool(name="qnp", bufs=2) as qnpool,
                tc.tile_pool(name="ep", bufs=2) as epool,
                tc.tile_pool(name="dp", bufs=2) as dppool,
                tc.tile_pool(name="op", bufs=3) as opool,
                tc.tile_pool(name="ps_small", bufs=2, space="PSUM") as ps_small,
                tc.tile_pool(name="ps_trep", bufs=2, space="PSUM") as ps_trep,
                tc.tile_pool(name="ps_sc", bufs=2, space="PSUM") as ps_sc,
                tc.tile_pool(name="ps_out", bufs=2, space="PSUM") as ps_out,
            ):
                for g in range(nst):
                    b = g * F
                    qTt = qpool.tile([C, F], f32, tag="q")
                    nc.sync.dma_start(out=qTt, in_=qT[:, b : b + F])
                    mTt = mpool.tile([N, F], f32, tag="m")
                    nc.sync.dma_start(out=mTt, in_=mT[:, b : b + F])

                    # per-row per-head inverse-norm temperature, transposed [4, F]
                    qT2 = q2pool.tile([C, F], f32, tag="q2")
                    nc.gpsimd.tensor_mul(out=qT2, in0=qTt[:, :], in1=qTt[:, :])
                    ssT = ps_small.tile([NH, F], f32, tag="small")
                    nc.tensor.matmul(
                        ssT[:, :], onesblk[:, :], qT2[:, :], start=True, stop=True
                    )
                    lnss = lpool.tile([NH, F], f32, tag="ln")
                    nc.scalar.activation(out=lnss, in_=ssT[:, :], func=AF.Ln)
                    tT2 = tpool.tile([NH, F], f32, tag="t")
                    # exp(-0.5*ln(ss) + ln(scale_h)) = scale_h / sqrt(ss)
                    nc.scalar.activation(
                        out=tT2,
                        in_=lnss[:, :],
                        func=AF.Exp,
                        scale=-0.5,
                        bias=lnscale[:, :],
                    )
                    tRep = ps_trep.tile([C, F], f32, tag="trep")
                    nc.tensor.matmul(
                        tRep[:, :], blkind[:, :], tT2[:, :], start=True, stop=True
                    )
                    qnT = qnpool.tile([C, F], f32, tag="qn")
                    nc.vector.tensor_mul(out=qnT, in0=qTt[:, :], in1=tRep[:, :])

                    # scores^T [64, F] = knb^T @ qnT + replicated mask
                    scT = ps_sc.tile([4 * N, F], f32, tag="sc")
                    nc.tensor.matmul(
                        scT[:, :], knb[:, :], qnT[:, :], start=True, stop=False
                    )
                    for s in range(4):
                        nc.tensor.matmul(
                            scT[:, 128 * s : 128 * (s + 1)],
                            repl16[:, :],
                            mTt[:, 128 * s : 128 * (s + 1)],
                            start=False,
                            stop=(s == 3),
                        )
                    escT = epool.tile([4 * N, F], f32, tag="esc")
                    nc.scalar.activation(
                        out=escT, in_=scT[:, :], func=AF.Exp, bias=negscale[:, :]
                    )

                    # PV and denominator, row-major outputs
                    outP = ps_out.tile([C, F], f32, tag="o")
                    dnP = ps_small.tile([C, NH * 4], f32, tag="small")
                    for s in range(4):
                        nc.tensor.matmul(
                            outP[:, 128 * s : 128 * (s + 1)],
                            escT[:, 128 * s : 128 * (s + 1)],
                            vmap[:, :],
                            start=True,
                            stop=True,
                        )
                        nc.tensor.matmul(
                            dnP[:, NH * s : NH * (s + 1)],
                            escT[:, 128 * s : 128 * (s + 1)],
                            ones16[:, :],
                            start=True,
                            stop=True,
                        )
                    dinv = dppool.tile([C, NH * 4], f32, tag="dinv")
                    nc.vector.reciprocal(out=dinv, in_=dnP[:, :])
                    outF = opool.tile([C, F], f32, tag="of")
                    nc.vector.tensor_mul(
                        out=outF[:, :].rearrange("p (s h c) -> p s h c", s=4, h=NH),
                        in0=outP[:, :].rearrange("p (s h c) -> p s h c", s=4, h=NH),
                        in1=dinv[:, :]
                        .rearrange("p (s h) -> p s h", s=4)
                        .to_broadcast([C, 4, NH, HD]),
                    )
                    dst = out_d[b : b + F, :].rearrange("(s p) c -> p s c", p=128)
                    nc.sync.dma_start(
                        out=dst, in_=outF[:, :].rearrange("p (s c) -> p s c", s=4)
                    )
    return nc


def make_host_consts(nwin_total, logit_scale_np):
    """Host-precomputed constant tensors shared by all cores."""
    sel = np.zeros((C, 4 * N), np.float32)
    w = np.float32(1.0 / nwin_total)
    for r in range(C):
        for a in range(4):
            sel[r, N * a + (a * 4 + r % 4)] = w
    onesblk = np.zeros((C, NH), np.float32)
    for c in range(C):
        onesblk[c, c // HD] = 1.0
    blkind = np.ascontiguousarray(onesblk.T)
    repl16 = np.zeros((N, 4 * N), np.float32)
    for h in range(NH):
        repl16[:, N * h : N * (h + 1)] = np.eye(N, dtype=np.float32)
    ones16 = np.zeros((4 * N, NH), np.float32)
    for h in range(NH):
        ones16[N * h : N * (h + 1), h] = 1.0
    ident = np.eye(N, dtype=np.float32)
    lnsc = np.minimum(logit_scale_np.reshape(NH).astype(np.float32), LN100)
    lnscale = lnsc.reshape(NH, 1).astype(np.float32)
    negscale = np.repeat(-np.exp(lnsc), N).reshape(4 * N, 1).astype(np.float32)
    return {
        "sel": sel,
        "onesblk": onesblk,
        "blkind": blkind,
        "repl16": repl16,
        "ones16": ones16,
        "ident": ident,
        "lnscale": lnscale,
        "negscale": negscale,
    }


def prepare_run(q2, k2, v2, m2, logit_scale_np, ncores=NCORES):
    """Build the compiled Bass module and per-core input maps."""
    import concourse.bacc as bacc

    Q = q2.shape[0]
    rows = Q // ncores
    nwin = Q // N
    consts = make_host_consts(nwin, logit_scale_np)

    qT_full = np.ascontiguousarray(q2.T)
    mT_full = np.ascontiguousarray(m2.T)

    nc = bacc.Bacc(
        "TRN2",
        target_bir_lowering=False,
        debug=False,
        enable_asserts=True,
        num_devices=ncores,
    )
    build_kernel(nc, rows, ncores, nwin, use_collective=(ncores > 1))
    nc.compile()

    in_maps = []
    for cid in range(ncores):
        r0, r1 = cid * rows, (cid + 1) * rows
        m = {
            "qT": np.ascontiguousarray(qT_full[:, r0:r1]),
            "mT": np.ascontiguousarray(mT_full[:, r0:r1]),
            "kin": k2[r0:r1],
            "vin": v2[r0:r1],
        }
        m.update(consts)
        in_maps.append(m)
    return nc, in_maps


def run_device(q2, k2, v2, m2, logit_scale_np, ncores=NCORES, trace=False, tmpdir=None):
    """q2,k2,v2: (Q,C) f32; m2: (Q,N) f32. Returns (out (Q,C), results obj)."""
    from concourse.bass_utils import run_bass_kernel_spmd

    nc, in_maps = prepare_run(q2, k2, v2, m2, logit_scale_np, ncores)
    res = run_bass_kernel_spmd(
        nc, in_maps, list(range(ncores)), trace=trace, tmpdir=tmpdir
    )
    out = np.concatenate([res.results[cid]["out"] for cid in range(ncores)], axis=0)
    return out, res


def kernel(q, k, v, H, W, mask, logit_scale):
    q2 = np.asarray(q, np.float32).reshape(-1, C)
    k2 = np.asarray(k, np.float32).reshape(-1, C)
    v2 = np.asarray(v, np.float32).reshape(-1, C)
    m2 = np.asarray(mask, np.float32).reshape(-1, N)
    ls = np.asarray(logit_scale, np.float32)
    out, _ = run_device(q2, k2, v2, m2, ls)
    return out.reshape(q2.shape[0], 1, C).astype(np.float32)


if __name__ == "__main__":
    pass


# revision 13
# speedup vs baseline: 1.5631x; 1.5631x over previous
"""Trainium2 Bass kernel for windowed mean-pooled cosine-sim multi-head attention.

Math (matches reference):
  km[n,:]  = mean over 16384 windows of window-partitioned k (n in 0..15), same for v
  kn       = l2norm(km per head-slice of 32 ch)
  t[i,h]   = scale_h / ||q[i, head h]||          (scale_h = exp(min(logit_scale, ln 100)))
  s[i,h,n] = t[i,h] * (q[i,h,:] . kn[h,n,:]) + mask[i,n]
  out[i, h-slice] = softmax_n(s[i,h,:]) @ vm[h,n,:]

Sharding: q/mask/out row-sharded 8 ways; k/v row-sharded for the mean
reduction with an AllReduce of the tiny [32,128] partial means.

Host-side prep: q and mask are transposed on host (channel-major) so the
device can DMA channel-major tiles directly (f32 DMA-transpose is not
supported by HW); all FLOPs and all large-tensor streaming happen on device.
"""

import math
import sys

import numpy as np

sys.path.insert(0, "/opt/trn_rl_repo")

NCORES = 8
Q_TOTAL = 262144
C = 128
NH = 4
HD = 32
N = 16  # window size 4x4
F = 512  # supertile: 512 q rows (4 x 128) / 512 k rows per step
LN100 = math.log(100.0)


def build_kernel(nc, rows, ncores, nwin_total, use_collective):
    """Trace the per-core kernel into `nc`. rows = per-core row count."""
    import concourse.bass as bass
    import concourse.tile as tile
    from concourse import mybir

    f32 = mybir.dt.float32
    f32r = mybir.dt.float32r
    AF = mybir.ActivationFunctionType
    ALU = mybir.AluOpType
    nst = rows // F
    assert rows % (4 * F) == 0, "need rows % 2048 == 0 so selector phase repeats"

    # ---- kernel I/O ----
    qT = nc.dram_tensor("qT", [C, rows], f32, kind="ExternalInput")
    mT = nc.dram_tensor("mT", [N, rows], f32, kind="ExternalInput")
    kin = nc.dram_tensor("kin", [rows, C], f32, kind="ExternalInput")
    vin = nc.dram_tensor("vin", [rows, C], f32, kind="ExternalInput")
    # host-precomputed constants
    sel_d = nc.dram_tensor("sel", [C, 4 * N], f32, kind="ExternalInput")
    onesblk_d = nc.dram_tensor("onesblk", [C, NH], f32, kind="ExternalInput")
    blkind_d = nc.dram_tensor("blkind", [NH, C], f32, kind="ExternalInput")
    repl16_d = nc.dram_tensor("repl16", [N, 4 * N], f32, kind="ExternalInput")
    ones16_d = nc.dram_tensor("ones16", [4 * N, NH], f32, kind="ExternalInput")
    ident_d = nc.dram_tensor("ident", [N, N], f32, kind="ExternalInput")
    lnscale_d = nc.dram_tensor("lnscale", [NH, 1], f32, kind="ExternalInput")
    negscale_d = nc.dram_tensor("negscale", [4 * N, 1], f32, kind="ExternalInput")
    out_d = nc.dram_tensor("out", [rows, C], f32, kind="ExternalOutput")

    with tile.TileContext(nc) as tc:
        with (
            tc.tile_pool(name="consts", bufs=1) as cpool,
            tc.tile_pool(name="smalls", bufs=1) as spool,
            tc.tile_pool(name="dram", bufs=1, space="DRAM") as dpool,
        ):
            # load constants
            sel = cpool.tile([C, 4 * N], f32r)
            nc.sync.dma_start(out=sel, in_=sel_d[:, :].bitcast(f32r))
            onesblk = cpool.tile([C, NH], f32r)
            nc.sync.dma_start(out=onesblk, in_=onesblk_d[:, :].bitcast(f32r))
            blkind = cpool.tile([NH, C], f32r)
            nc.sync.dma_start(out=blkind, in_=blkind_d[:, :].bitcast(f32r))
            repl16 = cpool.tile([N, 4 * N], f32r)
            nc.sync.dma_start(out=repl16, in_=repl16_d[:, :].bitcast(f32r))
            ones16 = cpool.tile([4 * N, NH], f32r)
            nc.sync.dma_start(out=ones16, in_=ones16_d[:, :].bitcast(f32r))
            ident = cpool.tile([N, N], f32)
            nc.sync.dma_start(out=ident, in_=ident_d[:, :])
            lnscale = cpool.tile([NH, 1], f32)
            nc.sync.dma_start(out=lnscale, in_=lnscale_d[:, :])
            negscale = cpool.tile([4 * N, 1], f32)
            nc.sync.dma_start(out=negscale, in_=negscale_d[:, :])

            # ---------------- phase KV: windowed means ----------------
            with (
                tc.tile_pool(name="kv", bufs=6) as kvpool,
                tc.tile_pool(name="pkv", bufs=1, space="PSUM") as pkv,
            ):
                kmP = pkv.tile([N, 4 * C], f32, tag="km")
                vmP = pkv.tile([N, 4 * C], f32, tag="vm")
                for j in range(nst):
                    src_k = kin[j * F : (j + 1) * F, :].rearrange(
                        "(s p) c -> p s c", p=128
                    )
                    kst = kvpool.tile([128, 4, 128], f32r, tag="kst")
                    nc.sync.dma_start(out=kst, in_=src_k.bitcast(f32r))
                    src_v = vin[j * F : (j + 1) * F, :].rearrange(
                        "(s p) c -> p s c", p=128
                    )
                    vst = kvpool.tile([128, 4, 128], f32r, tag="vst")
                    nc.sync.dma_start(out=vst, in_=src_v.bitcast(f32r))
                    a = j % 4
                    first = j == 0
                    last = j == nst - 1
                    selr = sel[:, N * a : N * (a + 1)]
                    nc.tensor.matmul(
                        kmP[:, :],
                        selr,
                        kst[:, :, :],
                        start=first,
                        stop=last,
                    )
                    nc.tensor.matmul(
                        vmP[:, :],
                        selr,
                        vst[:, :, :],
                        start=first,
                        stop=last,
                    )
                # sum the 4 sub-position partials: [16,(s4)(c128)] -> [16,128]
                km_part = spool.tile([N, C], f32)
                nc.vector.reduce_sum(
                    out=km_part,
                    in_=kmP[:, :].rearrange("p (s c) -> p c s", s=4),
                    axis=mybir.AxisListType.X,
                )
                vm_part = spool.tile([N, C], f32)
                nc.vector.reduce_sum(
                    out=vm_part,
                    in_=vmP[:, :].rearrange("p (s c) -> p c s", s=4),
                    axis=mybir.AxisListType.X,
                )

            if use_collective:
                cc_in = dpool.tile([2 * N, C], f32)
                cc_out = dpool.tile([2 * N, C], f32)
                nc.gpsimd.dma_start(out=cc_in[0:N, :], in_=km_part[:, :])
                nc.gpsimd.dma_start(out=cc_in[N : 2 * N, :], in_=vm_part[:, :])
                nc.gpsimd.collective_compute(
                    "AllReduce",
                    mybir.AluOpType.add,
                    replica_groups=[list(range(ncores))],
                    ins=[cc_in.opt()],
                    outs=[cc_out.opt()],
                )
                km = spool.tile([N, C], f32)
                nc.sync.dma_start(out=km, in_=cc_out[0:N, :])
                vm = spool.tile([N, C], f32)
                nc.sync.dma_start(out=vm, in_=cc_out[N : 2 * N, :])
            else:
                km, vm = km_part, vm_part

            # ---------------- build knb (normalized kT, block-diag) & vmap ----
            kms = spool.tile([N, C], f32)
            nc.scalar.activation(out=kms, in_=km[:, :], func=AF.Square)
            ssk = spool.tile([N, NH], f32)
            nc.vector.reduce_sum(
                out=ssk,
                in_=kms[:, :].rearrange("p (h c) -> p h c", h=NH),
                axis=mybir.AxisListType.X,
            )
            sqk = spool.tile([N, NH], f32)
            nc.scalar.activation(out=sqk, in_=ssk[:, :], func=AF.Sqrt)
            tK = spool.tile([N, NH], f32)
            nc.vector.reciprocal(out=tK, in_=sqk[:, :])
            kn = spool.tile([N, C], f32)
            for h in range(NH):
                nc.vector.tensor_scalar_mul(
                    kn[:, HD * h : HD * (h + 1)],
                    km[:, HD * h : HD * (h + 1)],
                    tK[:, h : h + 1],
                )
            knb = cpool.tile([C, 4 * N], f32r)
            nc.gpsimd.memset(knb[:, :].bitcast(f32), 0.0)
            with tc.tile_pool(name="pknb", bufs=1, space="PSUM") as pknb:
                knbP = pknb.tile([HD, 4 * N], f32)
                for h in range(NH):
                    nc.tensor.transpose(
                        knbP[:, N * h : N * (h + 1)],
                        kn[:, HD * h : HD * (h + 1)],
                        ident[:, :],
                    )
                knb_stage = spool.tile([HD, 4 * N], f32)
                nc.scalar.copy(out=knb_stage, in_=knbP[:, :])
                for h in range(NH):
                    nc.sync.dma_start(
                        out=knb[HD * h : HD * (h + 1), N * h : N * (h + 1)],
                        in_=knb_stage[:, N * h : N * (h + 1)].bitcast(f32r),
                    )
            vmap = cpool.tile([4 * N, C], f32r)
            nc.gpsimd.memset(vmap[:, :].bitcast(f32), 0.0)
            for h in range(NH):
                nc.sync.dma_start(
                    out=vmap[N * h : N * (h + 1), HD * h : HD * (h + 1)],
                    in_=vm[:, HD * h : HD * (h + 1)].bitcast(f32r),
                )

            # ---------------- phase Q ----------------
            with (
                tc.tile_pool(name="qp", bufs=4) as qpool,
                tc.tile_pool(name="mp", bufs=4) as mpool,
                tc.tile_pool(name="q2p", bufs=2) as q2pool,
                tc.tile_pool(name="lp", bufs=2) as lpool,
                tc.tile_pool(name="tp", bufs=2) as tpool,
                tc.tile_pool(name="qnp", bufs=2) as qnpool,
                tc.tile_pool(name="ep", bufs=2) as epool,
                tc.tile_pool(name="dp", bufs=2) as dppool,
                tc.tile_pool(name="op", bufs=3) as opool,
                tc.tile_pool(name="ps_small", bufs=2, space="PSUM") as ps_small,
                tc.tile_pool(name="ps_trep", bufs=2, space="PSUM") as ps_trep,
                tc.tile_pool(name="ps_sc", bufs=2, space="PSUM") as ps_sc,
                tc.tile_pool(name="ps_out", bufs=2, space="PSUM") as ps_out,
            ):
                for g in range(nst):
                    b = g * F
                    qTt = qpool.tile([C, F], f32, tag="q")
                    nc.sync.dma_start(out=qTt, in_=qT[:, b : b + F])
                    mTt = mpool.tile([N, F], f32r, tag="m")
                    nc.sync.dma_start(out=mTt, in_=mT[:, b : b + F].bitcast(f32r))

                    # per-row per-head inverse-norm temperature, transposed [4, F]
                    qT2 = q2pool.tile([C, F], f32r, tag="q2")
                    nc.gpsimd.tensor_mul(out=qT2, in0=qTt[:, :], in1=qTt[:, :])
                    ssT = ps_small.tile([NH, F], f32, tag="small")
                    nc.tensor.matmul(
                        ssT[:, :],
                        onesblk[:, :],
                        qT2[:, :],
                        start=True,
                        stop=True,
                    )
                    lnss = lpool.tile([NH, F], f32, tag="ln")
                    nc.scalar.activation(out=lnss, in_=ssT[:, :], func=AF.Ln)
                    tT2 = tpool.tile([NH, F], f32r, tag="t")
                    # exp(-0.5*ln(ss) + ln(scale_h)) = scale_h / sqrt(ss)
                    nc.scalar.activation(
                        out=tT2,
                        in_=lnss[:, :],
                        func=AF.Exp,
                        scale=-0.5,
                        bias=lnscale[:, :],
                    )
                    tRep = ps_trep.tile([C, F], f32, tag="trep")
                    nc.tensor.matmul(
                        tRep[:, :],
                        blkind[:, :],
                        tT2[:, :],
                        start=True,
                        stop=True,
                    )
                    qnT = qnpool.tile([C, F], f32r, tag="qn")
                    nc.vector.tensor_mul(out=qnT, in0=qTt[:, :], in1=tRep[:, :])

                    # scores^T [64, F] = knb^T @ qnT + replicated mask
                    scT = ps_sc.tile([4 * N, F], f32, tag="sc")
                    nc.tensor.matmul(
                        scT[:, :],
                        knb[:, :],
                        qnT[:, :],
                        start=True,
                        stop=False,
                    )
                    nc.tensor.matmul(
                        scT[:, :],
                        repl16[:, :],
                        mTt[:, :],
                        start=False,
                        stop=True,
                    )
                    escT = epool.tile([4 * N, F], f32r, tag="esc")
                    nc.scalar.activation(
                        out=escT, in_=scT[:, :], func=AF.Exp, bias=negscale[:, :]
                    )

                    # PV and denominator, row-major outputs
                    outP = ps_out.tile([C, F], f32, tag="o")
                    dnP = ps_small.tile([C, NH * 4], f32, tag="small")
                    for s in range(4):
                        nc.tensor.matmul(
                            outP[:, 128 * s : 128 * (s + 1)],
                            escT[:, 128 * s : 128 * (s + 1)],
                            vmap[:, :],
                            start=True,
                            stop=True,
                        )
                        nc.tensor.matmul(
                            dnP[:, NH * s : NH * (s + 1)],
                            escT[:, 128 * s : 128 * (s + 1)],
                            ones16[:, :],
                            start=True,
                            stop=True,
                        )
                    dinv = dppool.tile([C, NH * 4], f32, tag="dinv")
                    nc.vector.reciprocal(out=dinv, in_=dnP[:, :])
                    outF = opool.tile([C, F], f32, tag="of")
                    nc.vector.tensor_mul(
                        out=outF[:, :].rearrange("p (s h c) -> p s h c", s=4, h=NH),
                        in0=outP[:, :].rearrange("p (s h c) -> p s h c", s=4, h=NH),
                        in1=dinv[:, :]
                        .rearrange("p (s h) -> p s h", s=4)
                        .to_broadcast([C, 4, NH, HD]),
                    )
                    dst = out_d[b : b + F, :].rearrange("(s p) c -> p s c", p=128)
                    nc.sync.dma_start(
                        out=dst, in_=outF[:, :].rearrange("p (s c) -> p s c", s=4)
                    )
    return nc


def make_host_consts(nwin_total, logit_scale_np):
    """Host-precomputed constant tensors shared by all cores."""
    sel = np.zeros((C, 4 * N), np.float32)
    w = np.float32(1.0 / nwin_total)
    for r in range(C):
        for a in range(4):
            sel[r, N * a + (a * 4 + r % 4)] = w
    onesblk = np.zeros((C, NH), np.float32)
    for c in range(C):
        onesblk[c, c // HD] = 1.0
    blkind = np.ascontiguousarray(onesblk.T)
    repl16 = np.zeros((N, 4 * N), np.float32)
    for h in range(NH):
        repl16[:, N * h : N * (h + 1)] = np.eye(N, dtype=np.float32)
    ones16 = np.zeros((4 * N, NH), np.float32)
    for h in range(NH):
        ones16[N * h : N * (h + 1), h] = 1.0
    ident = np.eye(N, dtype=np.float32)
    lnsc = np.minimum(logit_scale_np.reshape(NH).astype(np.float32), LN100)
    lnscale = lnsc.reshape(NH, 1).astype(np.float32)
    negscale = np.repeat(-np.exp(lnsc), N).reshape(4 * N, 1).astype(np.float32)
    return {
        "sel": sel,
        "onesblk": onesblk,
        "blkind": blkind,
        "repl16": repl16,
        "ones16": ones16,
        "ident": ident,
        "lnscale": lnscale,
        "negscale": negscale,
    }


def prepare_run(q2, k2, v2, m2, logit_scale_np, ncores=NCORES):
    """Build the compiled Bass module and per-core input maps."""
    import concourse.bacc as bacc

    Q = q2.shape[0]
    rows = Q // ncores
    nwin = Q // N
    consts = make_host_consts(nwin, logit_scale_np)

    qT_full = np.ascontiguousarray(q2.T)
    mT_full = np.ascontiguousarray(m2.T)

    nc = bacc.Bacc(
        "TRN2",
        target_bir_lowering=False,
        debug=False,
        enable_asserts=True,
        num_devices=ncores,
    )
    build_kernel(nc, rows, ncores, nwin, use_collective=(ncores > 1))
    nc.compile()

    in_maps = []
    for cid in range(ncores):
        r0, r1 = cid * rows, (cid + 1) * rows
        m = {
            "qT": np.ascontiguousarray(qT_full[:, r0:r1]),
            "mT": np.ascontiguousarray(mT_full[:, r0:r1]),
            "kin": k2[r0:r1],
            "vin": v2[r0:r1],
        }
        m.update(consts)
        in_maps.append(m)
    return nc, in_maps


def run_device(q2, k2, v2, m2, logit_scale_np, ncores=NCORES, trace=False, tmpdir=None):
    """q2,k2,v2: (Q,C) f32; m2: (Q,N) f32. Returns (out (Q,C), results obj)."""
    from concourse.bass_utils import run_bass_kernel_spmd

    nc, in_maps = prepare_run(q2, k2, v2, m2, logit_scale_np, ncores)
    res = run_bass_kernel_spmd(
        nc, in_maps, list(range(ncores)), trace=trace, tmpdir=tmpdir
    )
    out = np.concatenate([res.results[cid]["out"] for cid in range(ncores)], axis=0)
    return out, res


def kernel(q, k, v, H, W, mask, logit_scale):
    q2 = np.asarray(q, np.float32).reshape(-1, C)
    k2 = np.asarray(k, np.float32).reshape(-1, C)
    v2 = np.asarray(v, np.float32).reshape(-1, C)
    m2 = np.asarray(mask, np.float32).reshape(-1, N)
    ls = np.asarray(logit_scale, np.float32)
    out, _ = run_device(q2, k2, v2, m2, ls)
    return out.reshape(q2.shape[0], 1, C).astype(np.float32)


if __name__ == "__main__":
    pass
